# revision 4
# baseline (speedup 1.0000x reference)
"""AmbientReflectionNet Trainium2 kernel (8 NeuronCores, data parallel).

Reference computation (per point):
  n = l2norm(normals); v = l2norm(view_dirs)
  visible = dot(n, v) > 0
  diffuse  = visible ? MLP_d(n)              : 0   (3->256->256->256->3, ReLU)
  specular = visible ? MLP_s([n,v,rough,r0]) : 0   (8->256->256->256->3, ReLU)

Fast path (all biases zero, which setup_inputs produces): the visibility
mask is folded into the normalized inputs -- with zero biases, masked
(zeroed) inputs propagate exact zeros through every ReLU layer, so no
output-side masking is needed.

Layout strategy per core (P/8 = 32768 points, 64 tiles of 512 points):
  - load point-major [128, 8, 8] tiles; normalize + mask on GPSIMD (idle
    engine) with one ACT Rsqrt; PE-transpose to feature-major [8, 512]
  - MLP layers as feature-major fp16 matmuls (1 col/cycle at free dim 512)
  - ReLU epilogues: half0 on ScalarE, half1 on VectorE (parallel latency)
  - layer 3 col-tiled: diffuse at PSUM partitions 0-3, specular at 32-35,
    concurrent on PE; single ACT copy [36,512] -> SBUF, 2 output DMAs
PSUM: mm ring-6 (12KB) + l3 (2KB) + transpose ptr (1KB) = 15KB, all
matmul targets bank-aligned by pool creation order.
"""

import numpy as np

import concourse.bass as bass
import concourse.mybir as mybir
import concourse.tile as tile
from concourse import bacc
from concourse.bass_utils import run_bass_kernel_spmd

NCORES = 8
P_FULL = 262144
PPC = P_FULL // NCORES  # points per core
TILE = 512
NT = PPC // TILE
H = 256
F32 = mybir.dt.float32
FP16 = mybir.dt.float16
EPS = 1e-12

_CACHE = {}


def _build_fast():
    from contextlib import ExitStack

    nc = bacc.Bacc()

    pts = nc.declare_dram_parameter("pts", [PPC, 8], F32, isOutput=False)
    identb_in = nc.declare_dram_parameter("identb", [128, 128], FP16, isOutput=False)

    # layer-0 weights, row-packed: rows 0-2 diffuse (n), rows 64-71
    # specular (n,v,ro,r0); [k, half, m]
    w0pack_in = nc.declare_dram_parameter("W0pack", [128, 2, 128], FP16, isOutput=False)
    dWp = {
        ("d", 1): nc.declare_dram_parameter("dW1p", [H, H], FP16, isOutput=False),
        ("s", 1): nc.declare_dram_parameter("sW1p", [H, H], FP16, isOutput=False),
        ("d", 2): nc.declare_dram_parameter("dW2p", [H, H], FP16, isOutput=False),
        ("s", 2): nc.declare_dram_parameter("sW2p", [H, H], FP16, isOutput=False),
        ("d", 3): nc.declare_dram_parameter("dW3p", [H, 4], FP16, isOutput=False),
        ("s", 3): nc.declare_dram_parameter("sW3p", [H, 4], FP16, isOutput=False),
    }

    out_d = nc.declare_dram_parameter("out_d", [3, PPC], F32, isOutput=True)
    out_s = nc.declare_dram_parameter("out_s", [3, PPC], F32, isOutput=True)

    with tile.TileContext(nc) as tc, ExitStack() as ctx:
        # PSUM pools -- creation order fixes addresses: mm ring-6 at banks
        # 0-5, l3 at bank 6, transpose ptr at bank 7 (all bank-aligned).
        ps_mm = ctx.enter_context(tc.tile_pool(name="psmm", bufs=6, space="PSUM"))
        ps_l3 = ctx.enter_context(tc.tile_pool(name="psl3", bufs=1, space="PSUM"))
        ps_tr = ctx.enter_context(tc.tile_pool(name="pstr", bufs=1, space="PSUM"))

        const = ctx.enter_context(tc.tile_pool(name="const", bufs=1))
        pool_araw = ctx.enter_context(tc.tile_pool(name="paraw", bufs=2))
        pool_in = ctx.enter_context(tc.tile_pool(name="pin", bufs=2))
        pool_rhs = ctx.enter_context(tc.tile_pool(name="prhs", bufs=3))
        pool_h = ctx.enter_context(tc.tile_pool(name="ph", bufs=2))
        pool_out = ctx.enter_context(tc.tile_pool(name="pout", bufs=3))

        # ---- constants ----
        identb = const.tile([128, 128], FP16)
        nc.sync.dma_start(identb, identb_in[:, :])

        W0pack = const.tile([128, 2, 128], FP16, name="W0pack")
        nc.sync.dma_start(W0pack, w0pack_in[:, :, :])

        Wmid = {}
        for pfx in ("d", "s"):
            for li in (1, 2):
                w = const.tile([128, 2, H], FP16, name=f"W{li}{pfx}")
                nc.sync.dma_start(w, dWp[pfx, li].rearrange("(c p) m -> p c m", p=128))
                Wmid[pfx, li] = w

        W3 = {}
        for pfx in ("d", "s"):
            w = const.tile([128, 2, 4], FP16, name=f"W3{pfx}")
            nc.sync.dma_start(w, dWp[pfx, 3].rearrange("(c p) m -> p c m", p=128))
            W3[pfx] = w

        # ---- warm-up: touch every const DMA from PE; also initialize the
        # whole l3 bank so the [36,512] epilogue copy never reads
        # never-written PSUM (rows 4-31 stay at identity values forever).
        ps3w = ps_l3.tile([128, 512], F32, tag="l3", name="ps3w")
        for k in range(4):
            nc.tensor.matmul(
                ps3w[:, k * 128 : (k + 1) * 128], identb, identb,
                start=True, stop=True,
            )
        warmset = [
            W0pack[:, 0, :],
            Wmid["d", 1][:, 0, 0:128],
            Wmid["s", 1][:, 0, 0:128],
            Wmid["d", 2][:, 0, 0:128],
            Wmid["s", 2][:, 0, 0:128],
            W3["d"][:, 0, :],
            W3["s"][:, 0, :],
        ]
        for wt in warmset:
            kp, fp = wt.shape
            wps = ps_mm.tile([128, 512], F32, tag="mm", name="wps")
            nc.tensor.matmul(
                wps[0:fp, 0:128], wt, identb[0:kp, :], start=True, stop=True
            )

        pts_pm2 = pts.rearrange("(t g p) c -> t p g c", p=128, g=8)

        for tp in range(NT // 2):
            # ---- load two tiles point-major [128, 8, 8] ----
            Araw = pool_araw.tile(
                [128, 8, 8], F32, tag="araw", name=f"araw{tp}"
            )
            nc.gpsimd.dma_start(Araw, pts_pm2[tp])

            # ---- normalize + visibility mask on GPSIMD ----
            # cols of Araw: n(3), v(3), ro, r0
            S = pool_in.tile([128, 8, 9], F32, tag="S", name="S")
            nc.gpsimd.tensor_tensor(
                S[:, :, 0:6], Araw[:, :, 0:6], Araw[:, :, 0:6], mybir.AluOpType.mult
            )
            nc.gpsimd.tensor_tensor(
                S[:, :, 6:9], Araw[:, :, 0:3], Araw[:, :, 3:6], mybir.AluOpType.mult
            )
            R = pool_in.tile([128, 8, 3], F32, tag="R", name="R")
            Sv = S.rearrange("p g (q c) -> p g q c", c=3)
            nc.gpsimd.tensor_tensor(
                R, Sv[:, :, :, 0], Sv[:, :, :, 1], mybir.AluOpType.add
            )
            nc.gpsimd.tensor_tensor(
                R, R, Sv[:, :, :, 2], mybir.AluOpType.add
            )
            # mask = (n.v raw) > 0  (sign identical to normalized dot)
            M8 = pool_in.tile([128, 8, 1], F32, tag="M8", name="M8")
            nc.gpsimd.tensor_scalar(
                M8, R[:, :, 2:3], 0.0, None, mybir.AluOpType.is_gt
            )
            nc.scalar.activation(
                R[:, :, 0:2], R[:, :, 0:2], mybir.ActivationFunctionType.Sqrt
            )
            nc.gpsimd.tensor_scalar_max(R[:, :, 0:2], R[:, :, 0:2], EPS)
            nc.vector.reciprocal(R[:, :, 0:2], R[:, :, 0:2])
            nc.gpsimd.tensor_tensor(
                R[:, :, 0:2],
                R[:, :, 0:2],
                M8.to_broadcast([128, 8, 2]),
                mybir.AluOpType.mult,
            )
            # A cols: n(3), v(3), ro, r0 -- all premultiplied by mask
            A = pool_in.tile([128, 8, 8], FP16, tag="A", name="A")
            nc.gpsimd.tensor_tensor(
                A[:, :, 0:3],
                Araw[:, :, 0:3],
                R[:, :, 0:1].to_broadcast([128, 8, 3]),
                mybir.AluOpType.mult,
            )
            nc.gpsimd.tensor_tensor(
                A[:, :, 3:6],
                Araw[:, :, 3:6],
                R[:, :, 1:2].to_broadcast([128, 8, 3]),
                mybir.AluOpType.mult,
            )
            nc.gpsimd.tensor_tensor(
                A[:, :, 6:8],
                Araw[:, :, 6:8],
                M8.to_broadcast([128, 8, 2]),
                mybir.AluOpType.mult,
            )

            for u in range(2):
                t = 2 * tp + u
                ptr = ps_tr.tile([8, 512], FP16, tag="tr", name="ptr")
                for g in range(4):
                    nc.tensor.transpose(
                        ptr[:, g * 128 : (g + 1) * 128],
                        A[:, 4 * u + g, 0:8],
                        identb,
                    )
                rhs0 = pool_rhs.tile([72, 512], FP16, tag="rhs0", name="rhs0")
                nc.vector.tensor_copy(rhs0[0:8, :], ptr)
                nc.vector.tensor_copy(rhs0[64:72, :], rhs0[0:8, :])

                # ---- layer 0: diffuse rows 0-2 / specular rows 64-71 as
                # concurrent row-tiles of the PE array ----
                ps0 = {}
                for half in range(2):
                    ps_d = ps_mm.tile([128, 512], F32, tag="mm", name="psd")
                    ps_s = ps_mm.tile([128, 512], F32, tag="mm", name="pss")
                    nc.tensor.matmul(
                        ps_d, W0pack[0:3, half, :], rhs0[0:3, :],
                        start=True, stop=True, tile_position=(0, 0),
                    )
                    nc.tensor.matmul(
                        ps_s, W0pack[64:72, half, :], rhs0[64:72, :],
                        start=True, stop=True, tile_position=(64, 0),
                    )
                    ps0["d", half] = ps_d
                    ps0["s", half] = ps_s
                hcur = {}
                for pfx in ("d", "s"):
                    hn = pool_h.tile([128, 2, 512], FP16, tag=f"h1{pfx}")
                    nc.scalar.activation(
                        hn[:, 0, :], ps0[pfx, 0],
                        mybir.ActivationFunctionType.Relu,
                    )
                    nc.vector.tensor_scalar_max(hn[:, 1, :], ps0[pfx, 1], 0.0)
                    hcur[pfx] = hn

                # ---- layers 1, 2 ----
                for li in (1, 2):
                    hnext = {}
                    for pfx in ("d", "s"):
                        pss = []
                        for half in range(2):
                            ps = ps_mm.tile([128, 512], F32, tag="mm", name="ps")
                            for c in range(2):
                                nc.tensor.matmul(
                                    ps,
                                    Wmid[pfx, li][:, c, half * 128 : half * 128 + 128],
                                    hcur[pfx][:, c, :],
                                    start=(c == 0),
                                    stop=(c == 1),
                                )
                            pss.append(ps)
                        hn = pool_h.tile([128, 2, 512], FP16, tag=f"h{li + 1}{pfx}")
                        nc.scalar.activation(
                            hn[:, 0, :], pss[0],
                            mybir.ActivationFunctionType.Relu,
                        )
                        nc.vector.tensor_scalar_max(hn[:, 1, :], pss[1], 0.0)
                        hnext[pfx] = hn
                    hcur = hnext

                # ---- layer 3, col-tiled: d -> psum partitions 0-3,
                # s -> partitions 32-35, concurrent on PE ----
                ps3 = ps_l3.tile([128, 512], F32, tag="l3", name="ps3")
                for c in range(2):
                    nc.tensor.matmul(
                        ps3[0:4, :], W3["d"][:, c, :], hcur["d"][:, c, :],
                        start=(c == 0), stop=(c == 1),
                    )
                    nc.tensor.matmul(
                        ps3[32:36, :], W3["s"][:, c, :], hcur["s"][:, c, :],
                        start=(c == 0), stop=(c == 1),
                    )
                osb = pool_out.tile([36, 512], F32, tag="osb", name="osb")
                nc.scalar.activation(
                    osb, ps3[0:36, :], mybir.ActivationFunctionType.Copy
                )
                nc.sync.dma_start(out_d[:, t * TILE : (t + 1) * TILE], osb[0:3, :])
                nc.sync.dma_start(out_s[:, t * TILE : (t + 1) * TILE], osb[32:35, :])

    nc.compile()
    return nc


def _pack_weights_fast(inputs):
    w = {}
    pack = np.zeros((128, 2, 128), np.float32)
    d0 = np.asarray(inputs["dW0"], np.float32)  # [3, 256]
    s0 = np.asarray(inputs["sW0"], np.float32)  # [8, 256]
    for h in range(2):
        pack[0:3, h, :] = d0[:, h * 128 : h * 128 + 128]
        pack[64:72, h, :] = s0[:, h * 128 : h * 128 + 128]
    w["W0pack"] = pack.astype(np.float16)
    for pfx in ("d", "s"):
        for li in (1, 2):
            w[f"{pfx}W{li}p"] = np.asarray(inputs[f"{pfx}W{li}"], dtype=np.float16)
        w[f"{pfx}W3p"] = np.asarray(
            np.concatenate(
                [inputs[f"{pfx}W3"], np.zeros((H, 1), np.float32)], axis=1
            ),
            dtype=np.float16,
        )  # [H, 4]
    return w


# ---------------------------------------------------------------------------
# Safe fallback (nonzero biases): original baseline kernel.
# ---------------------------------------------------------------------------


def _build_safe():
    from contextlib import ExitStack

    nc = bacc.Bacc()

    pts = nc.declare_dram_parameter("pts", [PPC, 8], F32, isOutput=False)
    identb_in = nc.declare_dram_parameter("identb", [128, 128], FP16, isOutput=False)

    w0pack_in = nc.declare_dram_parameter("W0pack", [128, 2, 128], FP16, isOutput=False)
    dWp = {
        ("d", 1): nc.declare_dram_parameter("dW1p", [H, H], FP16, isOutput=False),
        ("s", 1): nc.declare_dram_parameter("sW1p", [H, H], FP16, isOutput=False),
        ("d", 2): nc.declare_dram_parameter("dW2p", [H, H], FP16, isOutput=False),
        ("s", 2): nc.declare_dram_parameter("sW2p", [H, H], FP16, isOutput=False),
        ("d", 3): nc.declare_dram_parameter("dW3p", [H, 4], FP16, isOutput=False),
        ("s", 3): nc.declare_dram_parameter("sW3p", [H, 4], FP16, isOutput=False),
    }
    dB = {}
    for pfx in ("d", "s"):
        for i in range(4):
            n = H if i < 3 else 3
            dB[pfx, i] = nc.declare_dram_parameter(
                f"{pfx}b{i}", [n], F32, isOutput=False
            )

    out_d = nc.declare_dram_parameter("out_d", [3, PPC], F32, isOutput=True)
    out_s = nc.declare_dram_parameter("out_s", [3, PPC], F32, isOutput=True)

    with tile.TileContext(nc) as tc, ExitStack() as ctx:
        const = ctx.enter_context(tc.tile_pool(name="const", bufs=1))
        pool_in = ctx.enter_context(tc.tile_pool(name="pin", bufs=3))
        pool_araw = ctx.enter_context(tc.tile_pool(name="paraw", bufs=1))
        pool_rhs = ctx.enter_context(tc.tile_pool(name="prhs", bufs=3))
        pool_h = ctx.enter_context(tc.tile_pool(name="ph", bufs=2))
        pool_out = ctx.enter_context(tc.tile_pool(name="pout", bufs=3))
        ps_tr = ctx.enter_context(tc.tile_pool(name="pstr", bufs=2, space="PSUM"))
        ps_mm = {
            "d": ctx.enter_context(tc.tile_pool(name="psmmd", bufs=2, space="PSUM")),
            "s": ctx.enter_context(tc.tile_pool(name="psmms", bufs=2, space="PSUM")),
        }
        ps_l3 = ctx.enter_context(tc.tile_pool(name="psl3", bufs=1, space="PSUM"))

        identb = const.tile([128, 128], FP16)
        nc.sync.dma_start(identb, identb_in[:, :])

        W0pack = const.tile([128, 2, 128], FP16, name="W0pack")
        nc.sync.dma_start(W0pack, w0pack_in[:, :, :])

        Wmid = {}
        for pfx in ("d", "s"):
            for li in (1, 2):
                w = const.tile([128, 2, H], FP16, name=f"W{li}{pfx}")
                nc.sync.dma_start(w, dWp[pfx, li].rearrange("(c p) m -> p c m", p=128))
                Wmid[pfx, li] = w

        W3 = {}
        for pfx in ("d", "s"):
            w = const.tile([128, 2, 4], FP16, name=f"W3{pfx}")
            nc.sync.dma_start(w, dWp[pfx, 3].rearrange("(c p) m -> p c m", p=128))
            W3[pfx] = w

        Bias = {}
        for pfx in ("d", "s"):
            for li in (0, 1, 2):
                b = const.tile([128, 2], F32, name=f"B{li}{pfx}")
                nc.sync.dma_start(b, dB[pfx, li].rearrange("(h p) -> p h", p=128))
                Bias[pfx, li] = b
            b = const.tile([3, 1], F32, name=f"B3{pfx}")
            nc.sync.dma_start(b, dB[pfx, 3].rearrange("(c o) -> c o", o=1))
            Bias[pfx, 3] = b

        warm = ps_l3.tile([128, 128], F32, tag="l3d", name="warm")
        nc.tensor.matmul(warm, identb, identb, start=True, stop=True)
        nc.tensor.matmul(warm, W0pack[:, 0, :], identb, start=True, stop=True)
        for wt in (
            Wmid["d", 1][:, 0, 0:128],
            Wmid["s", 1][:, 0, 0:128],
            Wmid["d", 2][:, 0, 0:128],
            Wmid["s", 2][:, 0, 0:128],
            W3["d"][:, 0, :],
            W3["s"][:, 0, :],
        ):
            kp, fp = wt.shape
            nc.tensor.matmul(
                warm[0:fp, :], wt, identb[0:kp, :], start=True, stop=True
            )

        def relu_epilogue(dst, psrc, bias_ap, key):
            use_act = not (
                (key[0] == "s" and key[2] == 0) or key == ("d", 1, 1)
            )
            if use_act:
                nc.scalar.activation(
                    dst, psrc, mybir.ActivationFunctionType.Relu, bias=bias_ap
                )
            else:
                nc.vector.tensor_scalar(
                    dst, psrc, bias_ap, 0.0, mybir.AluOpType.add, mybir.AluOpType.max
                )

        pts_pm2 = pts.rearrange("(t g p) c -> t p g c", p=128, g=8)
        for tp in range(NT // 2):
            Araw = pool_araw.tile(
                [128, 8, 8], F32, tag=f"araw{tp}", name=f"araw{tp}"
            )
            nc.gpsimd.dma_start(Araw, pts_pm2[tp])

            S = pool_in.tile([128, 8, 9], F32, name="S")
            nc.vector.tensor_tensor(
                S[:, :, 0:6], Araw[:, :, 0:6], Araw[:, :, 0:6], mybir.AluOpType.mult
            )
            nc.vector.tensor_tensor(
                S[:, :, 6:9], Araw[:, :, 0:3], Araw[:, :, 3:6], mybir.AluOpType.mult
            )
            R = pool_in.tile([128, 8, 3], F32, name="R")
            nc.vector.tensor_reduce(
                R,
                S.rearrange("p g (q c) -> p g q c", c=3),
                axis=mybir.AxisListType.X,
                op=mybir.AluOpType.add,
            )
            A = pool_in.tile([128, 8, 9], FP16, name="A")
            nc.vector.tensor_scalar(
                A[:, :, 0:1], R[:, :, 2:3], 0.0, None, mybir.AluOpType.is_gt
            )
            nc.scalar.activation(
                R[:, :, 0:2], R[:, :, 0:2], mybir.ActivationFunctionType.Sqrt
            )
            nc.vector.tensor_scalar_max(R[:, :, 0:2], R[:, :, 0:2], EPS)
            nc.vector.reciprocal(R[:, :, 0:2], R[:, :, 0:2])
            nc.vector.tensor_tensor(
                A[:, :, 1:4],
                Araw[:, :, 0:3],
                R[:, :, 0:1].to_broadcast([128, 8, 3]),
                mybir.AluOpType.mult,
            )
            nc.vector.tensor_tensor(
                A[:, :, 4:7],
                Araw[:, :, 3:6],
                R[:, :, 1:2].to_broadcast([128, 8, 3]),
                mybir.AluOpType.mult,
            )
            nc.vector.tensor_scalar_mul(A[:, :, 7:9], Araw[:, :, 6:8], 1.0)

            for u in range(2):
                t = 2 * tp + u
                ptr = ps_tr.tile([9, 512], FP16, tag="tr", name="ptr")
                for g in range(4):
                    nc.tensor.transpose(
                        ptr[:, g * 128 : (g + 1) * 128],
                        A[:, 4 * u + g, 0:9],
                        identb,
                    )
                rhs0 = pool_rhs.tile([73, 512], FP16, tag="rhs0")
                nc.vector.tensor_copy(rhs0[0:9, :], ptr)
                nc.vector.tensor_copy(rhs0[64:73, :], rhs0[0:9, :])

                mb = pool_rhs.tile([3, 512], FP16, tag="mb")
                nc.vector.stream_shuffle(mb, rhs0[0:3, :], [0] * 32)

                hcur = {}
                for pfx in ("d", "s"):
                    hcur[pfx] = pool_h.tile(
                        [128, 2, 512], FP16, tag=f"h1{pfx}", name=f"h1{pfx}"
                    )
                for half in range(2):
                    ps_d = ps_mm["d"].tile([128, 512], F32, tag="mm")
                    ps_s = ps_mm["s"].tile([128, 512], F32, tag="mm")
                    nc.tensor.matmul(
                        ps_d, W0pack[0:4, half, :], rhs0[0:4, :],
                        start=True, stop=True, tile_position=(0, 0),
                    )
                    nc.tensor.matmul(
                        ps_s, W0pack[64:73, half, :], rhs0[64:73, :],
                        start=True, stop=True, tile_position=(64, 0),
                    )
                    relu_epilogue(
                        hcur["d"][:, half, :], ps_d,
                        Bias["d", 0][:, half : half + 1], ("d", 0, half),
                    )
                    relu_epilogue(
                        hcur["s"][:, half, :], ps_s,
                        Bias["s", 0][:, half : half + 1], ("s", 0, half),
                    )

                for li in (1, 2):
                    hnext = {}
                    for pfx in ("d", "s"):
                        hn = pool_h.tile([128, 2, 512], FP16, tag=f"h{li + 1}{pfx}")
                        for half in range(2):
                            ps = ps_mm[pfx].tile([128, 512], F32, tag="mm")
                            for c in range(2):
                                nc.tensor.matmul(
                                    ps,
                                    Wmid[pfx, li][:, c, half * 128 : half * 128 + 128],
                                    hcur[pfx][:, c, :],
                                    start=(c == 0),
                                    stop=(c == 1),
                                )
                            relu_epilogue(
                                hn[:, half, :],
                                ps,
                                Bias[pfx, li][:, half : half + 1],
                                (pfx, li, half),
                            )
                        hnext[pfx] = hn
                    hcur = hnext

                for pfx, outbuf in (("d", out_d), ("s", out_s)):
                    ps3 = ps_l3.tile([4, 512], F32, tag=f"l3{pfx}")
                    for c in range(2):
                        nc.tensor.matmul(
                            ps3,
                            W3[pfx][:, c, :],
                            hcur[pfx][:, c, :],
                            start=(c == 0),
                            stop=(c == 1),
                        )
                    osb = pool_out.tile([3, 512], F32, tag=f"o{pfx}")
                    nc.vector.scalar_tensor_tensor(
                        osb,
                        ps3[0:3, :],
                        Bias[pfx, 3][:, 0:1],
                        mb,
                        mybir.AluOpType.add,
                        mybir.AluOpType.mult,
                    )
                    nc.sync.dma_start(outbuf[:, t * TILE : (t + 1) * TILE], osb)

    nc.compile()
    return nc


def _pack_weights_safe(inputs):
    w = {}
    z3 = np.zeros((1, H), np.float32)
    d0 = np.concatenate([z3, inputs["dW0"]], axis=0)  # [4, H]
    s0 = np.concatenate([z3, inputs["sW0"]], axis=0)  # [9, H]
    pack = np.zeros((128, 2, 128), np.float32)
    for h in range(2):
        pack[0:4, h, :] = d0[:, h * 128 : h * 128 + 128]
        pack[64:73, h, :] = s0[:, h * 128 : h * 128 + 128]
    w["W0pack"] = pack.astype(np.float16)
    for pfx in ("d", "s"):
        for li in (1, 2):
            w[f"{pfx}W{li}p"] = np.asarray(inputs[f"{pfx}W{li}"], dtype=np.float16)
        w[f"{pfx}W3p"] = np.asarray(
            np.concatenate(
                [inputs[f"{pfx}W3"], np.zeros((H, 1), np.float32)], axis=1
            ),
            dtype=np.float16,
        )  # [H, 4]
        for li in range(4):
            w[f"{pfx}b{li}"] = np.ascontiguousarray(
                inputs[f"{pfx}b{li}"], dtype=np.float32
            )
    return w


# ---------------------------------------------------------------------------
# Mode selection + harness API
# ---------------------------------------------------------------------------


def _fast_ok(inputs):
    try:
        return all(
            not np.any(np.asarray(inputs[f"{pfx}b{i}"]))
            for pfx in ("d", "s")
            for i in range(4)
        )
    except Exception:
        return False


def _mode(inputs=None):
    if inputs is None:
        return _CACHE.get("mode", "fast")
    return "fast" if _fast_ok(inputs) else "safe"


def get_nc(inputs=None):
    mode = _mode(inputs)
    _CACHE["mode"] = mode
    key = f"nc_{mode}"
    if key not in _CACHE:
        _CACHE[key] = _build_fast() if mode == "fast" else _build_safe()
    return _CACHE[key]


def make_shards(inputs):
    mode = _mode(inputs)
    _CACHE["mode"] = mode
    if mode == "fast":
        wpack = _pack_weights_fast(inputs)
    else:
        wpack = _pack_weights_safe(inputs)
    pts_all = np.ascontiguousarray(
        np.concatenate(
            [
                np.asarray(inputs["normals"], np.float32),
                np.asarray(inputs["view_dirs"], np.float32),
                np.asarray(inputs["roughness"], np.float32),
                np.asarray(inputs["r0"], np.float32),
            ],
            axis=1,
        )
    )
    ident = np.eye(128, dtype=np.float16)
    shards = []
    for i in range(NCORES):
        sl = slice(i * PPC, (i + 1) * PPC)
        m = {"pts": pts_all[sl], "identb": ident}
        m.update(wpack)
        shards.append(m)
    return shards


def gather_outputs(results):
    diff = np.concatenate([results[i]["out_d"] for i in range(NCORES)], axis=1).T
    spec = np.concatenate([results[i]["out_s"] for i in range(NCORES)], axis=1).T
    return (
        np.ascontiguousarray(diff.astype(np.float32)),
        np.ascontiguousarray(spec.astype(np.float32)),
    )


def kernel(**inputs):
    nc = get_nc(inputs)
    shards = make_shards(inputs)
    res = run_bass_kernel_spmd(nc, shards, core_ids=list(range(NCORES)))
    return gather_outputs(res.results)
